# revision 1
# baseline (speedup 1.0000x reference)
"""2-layer LSTM (B=256, T=2048, I=32, H=64) + linear head + clip on 8 TRN2
NeuronCores via Bass/Tile. Batch sharded 8 ways (32 seqs/core); weights
replicated; no cross-core communication. Final bias-add + clip run on host
(the device emits raw W_out @ h2); everything else on device.
"""
import numpy as np
from contextlib import ExitStack

import concourse.bass as bass
from concourse import bacc
import concourse.tile as tile
from concourse import mybir
from concourse._compat import with_exitstack
from concourse.bass_utils import run_bass_kernel_spmd

F32 = mybir.dt.float32
AF = mybir.ActivationFunctionType
ALU = mybir.AluOpType

H = 64
I = 32
B = 256
T = 2048
NCORES = 8
BC = B // NCORES
NS = 256
X_CHUNK = 32
Y_GROUP = 16

def pack_weights(W_ih0, W_hh0, b_ih0, b_hh0, W_ih1, W_hh1, b_ih1, b_hh1, W_out):
    """Pack weights into stationary lhsT layouts. Gate order i,f,g,o (64 each).
    Pair 'if' = gates 0:128, pair 'go' = gates 128:256 with g columns doubled."""
    b0 = (b_ih0 + b_hh0).astype(np.float32)
    b1 = (b_ih1 + b_hh1).astype(np.float32)

    def mk_l0(gsl, dbl_first64):
        out = np.zeros((97, 128), dtype=np.float32)
        out[0:64, :] = W_hh0[gsl, :].T
        out[64:96, :] = W_ih0[gsl, :].T
        out[96, :] = b0[gsl]
        if dbl_first64:
            out[:, 0:64] *= 2.0
        return out

    def mk_l1(gsl, dbl_first64):
        out = np.zeros((128, 128), dtype=np.float32)
        out[0:64, :] = W_ih1[gsl, :].T
        out[64:128, :] = W_hh1[gsl, :].T
        if dbl_first64:
            out[:, 0:64] *= 2.0
        return out

    l0if = mk_l0(slice(0, 128), False)
    l0go = mk_l0(slice(128, 256), True)
    l1if = mk_l1(slice(0, 128), False)
    l1go = mk_l1(slice(128, 256), True)

    b1v = np.zeros((2, 128), dtype=np.float32)
    b1v[0, :] = b1[0:128]
    b1v[1, :] = b1[128:256]
    b1v[1, 0:64] *= 2.0

    mask = np.zeros((2, 64), dtype=np.float32)
    mask[0, 0:32] = 1.0
    mask[1, 32:64] = 1.0

    woutT = np.ascontiguousarray(W_out.reshape(1, H).T)  # [64,1]
    return dict(l0if=l0if, l0go=l0go, l1if=l1if, l1go=l1go,
                b1v=b1v, mask=mask, wout=woutT)


@with_exitstack
def lstm_kernel(ctx: ExitStack, tc: tile.TileContext, outs, ins, T, b_out):
    nc = tc.nc
    xt, y = ins["xt"], outs["y"]
    assert T % X_CHUNK == 0 and T % Y_GROUP == 0 and NS % X_CHUNK == 0

    wpool = ctx.enter_context(tc.tile_pool(name="w", bufs=1))
    state = ctx.enter_context(tc.tile_pool(name="state", bufs=1))
    sp = ctx.enter_context(tc.tile_pool(name="sp", bufs=4))
    ps0 = ctx.enter_context(tc.tile_pool(name="ps0", bufs=3, space="PSUM"))
    ps1 = ctx.enter_context(tc.tile_pool(name="ps1", bufs=3, space="PSUM"))
    psy = ctx.enter_context(tc.tile_pool(name="psy", bufs=2, space="PSUM"))

    # weights
    w = {}
    for name, shape in [("l0if", [97, 128]), ("l0go", [97, 128]),
                        ("l1if", [128, 128]), ("l1go", [128, 128]),
                        ("b1v", [2, 128]), ("mask", [2, 64])]:
        w[name] = wpool.tile(shape, F32, tag=name, name=name)
        nc.sync.dma_start(w[name][:], ins[name][:])
    # wout lives at partitions 64:128 to match the h2 rows of R1
    wout_t = wpool.tile([128, 1], F32, tag="wout", name="wout_t")
    nc.sync.dma_start(wout_t[64:128, :], ins["wout"][:])
    w["wout"] = wout_t[64:128, :]
    bout_t = wpool.tile([1, 1], F32, tag="bout", name="bout_t")
    nc.vector.memset(bout_t[:], b_out)

    # persistent state
    R0 = state.tile([97, NS * BC], F32, tag="R0", name="R0")   # [x;1;a] slots
    R1 = state.tile([128, NS * BC], F32, tag="R1", name="R1")  # [a;h2] slots
    C0 = state.tile([128, BC], F32, tag="C0", name="C0")
    C1 = state.tile([128, BC], F32, tag="C1", name="C1")
    nc.vector.memset(R0[0:64, 0:BC], 0.0)           # a slot 0 = 0
    nc.gpsimd.memset(R0[96:97, :], 1.0)             # ones row (all slots)
    nc.vector.memset(R1[:, 0:BC], 0.0)              # slot 0: a/h2 = 0
    nc.vector.memset(C0[64:128, :], 0.0)
    nc.vector.memset(C1[64:128, :], 0.0)

    def slot(k):
        return (k % NS) * BC

    def dma_x_chunk(c):
        t0 = c * X_CHUNK
        if t0 >= T:
            return
        s0 = slot(t0)
        src = xt[:, t0:t0 + X_CHUNK, :].rearrange("p t b -> p (t b)")
        nc.sync.dma_start(R0[64:96, s0:s0 + X_CHUNK * BC], src)

    # prefetch first chunks
    dma_x_chunk(0)
    dma_x_chunk(1)

    def phase_mm_sigma(layer, k):
        sl = slot(k)
        if layer == 0:
            g = ps0.tile([128, 64], F32, tag="g0", name="g0")
            rhs = R0[:, sl:sl + BC]
            nc.tensor.matmul(g[:, 0:32], w["l0if"][:], rhs, start=True, stop=True)
            nc.tensor.matmul(g[:, 32:64], w["l0go"][:], rhs, start=True, stop=True)
            S = sp.tile([128, 64], F32, tag="S0", name="S0")
        else:
            g = ps1.tile([128, 64], F32, tag="g1", name="g1")
            rhs = R1[:, sl:sl + BC]
            nc.tensor.matmul(g[:, 0:64], w["b1v"][:], w["mask"][:],
                             start=True, stop=False)
            nc.tensor.matmul(g[:, 0:32], w["l1if"][:], rhs, start=False, stop=False)
            nc.tensor.matmul(g[:, 32:64], w["l1go"][:], rhs, start=False, stop=True)
            S = sp.tile([128, 64], F32, tag="S1", name="S1")
        nc.scalar.activation(S[:], g[:], AF.Sigmoid)
        return S

    def phase_elem(layer, k, S):
        sl = slot(k)
        sl1 = slot(k + 1)
        C = C0 if layer == 0 else C1
        p = sp.tile([128, BC], F32, tag=f"p{layer}", name=f"p{layer}")
        nc.vector.scalar_tensor_tensor(p[64:128, :], S[0:64, 32:64], 0.5,
                                       S[0:64, 0:32], ALU.subtract, ALU.mult)
        nc.gpsimd.tensor_tensor(C[64:128, :], S[64:128, 0:32], C[64:128, :], ALU.mult)
        nc.vector.scalar_tensor_tensor(C[64:128, :], p[64:128, :], 2.0, C[64:128, :],
                                       ALU.mult, ALU.add)
        tch = sp.tile([128, BC], F32, tag=f"tc{layer}", name=f"tc{layer}")
        nc.scalar.activation(tch[64:128, :], C[64:128, :], AF.Tanh)
        if layer == 0:
            nc.vector.tensor_tensor(R0[0:64, sl1:sl1 + BC], S[64:128, 32:64],
                                    tch[64:128, :], ALU.mult)
            nc.gpsimd.tensor_copy(R1[0:64, sl:sl + BC], R0[0:64, sl1:sl1 + BC])
        else:
            nc.gpsimd.tensor_tensor(R1[64:128, sl1:sl1 + BC], S[64:128, 32:64],
                                    tch[64:128, :], ALU.mult)

    def step_layer(layer, k):
        phase_elem(layer, k, phase_mm_sigma(layer, k))


    def emit_y_group(m):
        """y for steps t in [m*YG, (m+1)*YG): h2[t] lives at slot t+1."""
        t0 = m * Y_GROUP
        n = Y_GROUP * BC
        yp = psy.tile([1, n], F32, tag="yp", name="yp")
        s0 = slot(t0 + 1)
        if s0 + n <= NS * BC:
            nc.tensor.matmul(yp[:, :], w["wout"], R1[64:128, s0:s0 + n],
                             start=True, stop=True)
        else:
            n1 = NS * BC - s0
            nc.tensor.matmul(yp[:, 0:n1], w["wout"], R1[64:128, s0:s0 + n1],
                             start=True, stop=True)
            nc.tensor.matmul(yp[:, n1:n], w["wout"], R1[64:128, 0:n - n1],
                             start=True, stop=True)
        ysb = sp.tile([1, n], F32, tag="ysb", name="ysb")
        nc.vector.tensor_copy(ysb[:], yp[:])
        nc.sync.dma_start(y[t0:t0 + Y_GROUP, :].rearrange("t b -> (t b)")[None, :],
                          ysb[:])

    # prologue: layer0 step 0
    step_layer(0, 0)
    # main: iteration k: layer0 step k, layer1 step k-1
    for k in range(1, T):
        if k % X_CHUNK == 0:
            dma_x_chunk(k // X_CHUNK + 1)
        S0 = phase_mm_sigma(0, k)
        S1 = phase_mm_sigma(1, k - 1)
        phase_elem(0, k, S0)
        phase_elem(1, k - 1, S1)
        if k % Y_GROUP == 0 and k >= Y_GROUP:
            emit_y_group(k // Y_GROUP - 1)
    # epilogue: layer1 step T-1
    step_layer(1, T - 1)
    emit_y_group(T // Y_GROUP - 1)


def build_nc(b_out):
    nc = bacc.Bacc("TRN2", target_bir_lowering=False, debug=False,
                   enable_asserts=False, num_devices=NCORES)
    ins = {
        "xt": nc.dram_tensor("xt", [I, T, BC], F32, kind="ExternalInput").ap(),
        "l0if": nc.dram_tensor("l0if", [97, 128], F32, kind="ExternalInput").ap(),
        "l0go": nc.dram_tensor("l0go", [97, 128], F32, kind="ExternalInput").ap(),
        "l1if": nc.dram_tensor("l1if", [128, 128], F32, kind="ExternalInput").ap(),
        "l1go": nc.dram_tensor("l1go", [128, 128], F32, kind="ExternalInput").ap(),
        "b1v": nc.dram_tensor("b1v", [2, 128], F32, kind="ExternalInput").ap(),
        "mask": nc.dram_tensor("mask", [2, 64], F32, kind="ExternalInput").ap(),
        "wout": nc.dram_tensor("wout", [H, 1], F32, kind="ExternalInput").ap(),
    }
    outs = {"y": nc.dram_tensor("y", [T, BC], F32, kind="ExternalOutput").ap()}
    with tile.TileContext(nc) as tc:
        lstm_kernel(tc, outs, ins, T=T, b_out=float(b_out))
    nc.compile()
    return nc


def shard_inputs(inputs):
    x = np.asarray(inputs["x"], dtype=np.float32)
    wk = pack_weights(*[np.asarray(inputs[k], dtype=np.float32) for k in
                        ["W_ih0", "W_hh0", "b_ih0", "b_hh0",
                         "W_ih1", "W_hh1", "b_ih1", "b_hh1", "W_out"]])
    in_maps = []
    for c in range(NCORES):
        xs = x[c * BC:(c + 1) * BC]
        m = dict(wk)
        m["xt"] = np.ascontiguousarray(xs.transpose(2, 1, 0))
        in_maps.append(m)
    return in_maps


def run(inputs, **kwargs):
    in_maps = shard_inputs(inputs)
    b_out = float(np.asarray(inputs["b_out"]).reshape(-1)[0])
    nc = build_nc(b_out)
    res = run_bass_kernel_spmd(nc, in_maps, core_ids=list(range(NCORES)), **kwargs)
    ys = []
    for r in res.results:
        yc = np.clip(r["y"].astype(np.float64) + b_out, 0.0, 1.0).astype(np.float32)
        ys.append(np.ascontiguousarray(yc.T)[:, :, None])
    return np.concatenate(ys, axis=0), res


def kernel(**inputs) -> np.ndarray:
    y, _ = run(inputs)
    return y

